# revision 1
# baseline (speedup 1.0000x reference)
"""Trainium2 Bass kernel for nn_MCPBRNN_SW_Variant_Routing_Norm.

Reference semantics: a single scalar nonlinear recurrence over the flattened
sequence u = x[time_lag:].reshape(-1) (length N = (B-time_lag)*T):

    c_{g+1} = f(c_g) * c_g + u_g,   f(c) = 1 - oo1 * sigmoid(w*c + b0)

with outputs recorded at the last step of each row i (global step
s_i = i*T + T-1): (oo*c, c, oo, 1-oo) evaluated at the carry-in state
c_{s_i}.  oo1, w, b0 are scalars derived from the (scalar) weights.

Key numerical structure exploited here: f in [0.73, 0.79] along the whole
trajectory, so the recurrence contracts with rate ~0.75/step -> state has
finite memory (~64 steps to fp32 precision).  Each of the 62 outputs is
therefore computed independently from an L-step window ending at its
output point, starting from c=0.  Windows live one-per-partition in a
[62, L] SBUF tile.

Within a window the recurrence is solved by Picard iteration on the
sequence space: given the previous iterate's c sequence, f_t is computed
in bulk (ACT sigmoid + ACT affine), and the linear recurrence
c_t = f_t*c_{t-1} + u_t is evaluated by the hardware scan instruction
(tensor_tensor_scan).  Convergence is geometric (x0.12/iteration), so a
handful of iterations reaches fp32 accuracy with ~4 large-tile
instructions each -- no per-step serial chain.

Hardware quirk: TensorScalarPtr-encoded DVE ops (the scan, tensor_scalar)
cannot carry sync-wait commands in this walrus codegen, so every
cross-engine dependency of a scan is absorbed by a tiny plain DVE op
right before it, and all scalar-affine work runs as ScalarE activations.

Sharding across the 8 cores: the problem is a single sequential recurrence
(see sharding hint) -- parameters and inputs are replicated; every core
runs the identical tiny computation and core 0's output is used.
"""

import numpy as np

_CACHE = {}


def _build(B, T, time_lag, L, K, w, b0, oo1, f0):
    import concourse.bacc as bacc
    import concourse.bass as bass  # noqa: F401
    import concourse.mybir as mybir
    from concourse.tile import TileContext

    f32 = mybir.dt.float32
    R = B - time_lag
    mult = mybir.AluOpType.mult
    add = mybir.AluOpType.add
    Sigmoid = mybir.ActivationFunctionType.Sigmoid
    Identity = mybir.ActivationFunctionType.Identity
    Copy = mybir.ActivationFunctionType.Copy

    # Bacc (not raw Bass): its finalize pipeline runs generate_event_semaphores,
    # which splits multi-wait sync lists to satisfy the 1-wait-per-instruction cap.
    nc = bacc.Bacc()
    x = nc.dram_tensor("x", [B, T], f32, kind="ExternalInput")
    out = nc.dram_tensor("out", [R, 4], f32, kind="ExternalOutput")

    # Wait-budget discipline: on this toolchain every instruction can carry
    # at most ONE sync wait (EventSemaphore: two), and Tile emits
    # semaphore waits for same-engine hazards too (deep pipelines) without
    # splitting overfull wait lists.  So the program is structured such
    # that no instruction ever needs more than one wait: at each
    # DVE<->ACT stream junction a tiny op writing a FRESH scratch tile
    # (no same-engine hazard) absorbs the single cross-engine wait, and
    # every subsequent op's remaining deps are same-engine (one self-wait)
    # or already covered by the engine's observed vector clock.
    with TileContext(nc) as tc:
        with tc.tile_pool(name="pool", bufs=1) as pool:
            u = pool.tile([R, L], f32)
            # window for output i: u indices T-1-L .. T-2 of row time_lag+i.
            # gpsimd (SWDGE) keeps this DMA's completion sem on the Pool
            # drain lane; the output DMA uses HWDGE/SP -- so each engine's
            # kernel-tail Drain waits on exactly one DMA sem (1-wait cap).
            nc.gpsimd.dma_start(out=u[:, :], in_=x[time_lag:B, T - 1 - L : T - 1])

            f = pool.tile([R, L], f32)
            sig = pool.tile([R, L], f32)
            c = pool.tile([R, L], f32)

            # activation() requires an AP bias; hold b0 in a [R,1] tile
            b0t = pool.tile([R, 1], f32)
            nc.vector.memset(b0t[:, :], b0)

            # Picard iteration 0 starts from the c=0 sequence: f == f(0) everywhere.
            nc.vector.memset(f[:, :], f0)

            for k in range(K):
                # DVE-side junction: absorb the one cross-engine wait
                # (k=0: u's DMA; k>0: ACT's f-update) into a scratch write.
                dscr = pool.tile([R, 1], f32, tag=f"dscr{k}")
                if k == 0:
                    nc.vector.tensor_copy(dscr[:, :], u[:, 0:1])
                else:
                    nc.vector.tensor_copy(dscr[:, :], f[:, 1:2])
                # c_t = f_t * c_{t-1} + u_t along the free dim, c_{-1} = 0.
                # Remaining deps are same-engine (one DVE self-wait).
                nc.vector.tensor_tensor_scan(
                    out=c[:, :], data0=f[:, :], data1=u[:, :],
                    initial=0.0, op0=mult, op1=add,
                )
                if k < K - 1:
                    # ACT-side junction: absorb the DVE wait (scan wrote c).
                    ascr = pool.tile([R, 1], f32, tag=f"ascr{k}")
                    nc.scalar.activation(out=ascr[:, :], in_=c[:, 0:1], func=Copy)
                    # f_t = 1 - oo1*sigmoid(w*c_{t-1} + b0); column 0 stays f(0)
                    nc.scalar.activation(
                        out=sig[:, 1:L], in_=c[:, 0 : L - 1],
                        func=Sigmoid, bias=b0t[:, :], scale=w,
                    )
                    nc.scalar.activation(
                        out=f[:, 1:L], in_=sig[:, 1:L],
                        func=Identity, bias=1.0, scale=-oo1,
                    )

            # Final outputs from C = carry-in state at the output step.
            # All on ScalarE; the res[:,1:2] copy doubles as the ACT-side
            # junction (fresh tile, absorbs the DVE wait on the last scan).
            res = pool.tile([R, 4], f32)
            sigf = pool.tile([R, 1], f32)
            C = c[:, L - 1 : L]
            nc.scalar.activation(out=res[:, 1:2], in_=C, func=Copy)                           # c
            nc.scalar.activation(out=sigf[:, :], in_=C, func=Sigmoid, bias=b0t[:, :], scale=w)
            nc.scalar.activation(out=res[:, 2:3], in_=sigf[:, :], func=Copy, scale=oo1)       # oo
            nc.scalar.activation(out=res[:, 0:1], in_=res[:, 2:3], func=Copy, scale=C)        # h = oo*C
            nc.scalar.activation(out=res[:, 3:4], in_=res[:, 2:3], func=Identity, bias=1.0, scale=-1.0)  # 1-oo
            nc.sync.dma_start(out=out[:, :], in_=res[:, :])

    nc.finalize()
    return nc


def run(inputs, trace=False, L=96, K=12):
    from concourse.bass_utils import run_bass_kernel_spmd

    x = np.ascontiguousarray(np.asarray(inputs["x"], dtype=np.float32))
    time_lag = int(inputs["time_lag"])
    p_norm = float(np.asarray(inputs["p_norm"]).reshape(-1)[0])
    w_r_yom = float(np.asarray(inputs["w_r_yom"]).reshape(-1)[0])
    w_r_yfm = float(np.asarray(inputs["w_r_yfm"]).reshape(-1)[0])
    b0 = float(np.asarray(inputs["b0_yom"]).reshape(-1)[0])
    w_b1 = float(np.asarray(inputs["w_b1_yom"]).reshape(-1)[0])

    oo1 = float(np.exp(w_r_yom) / (np.exp(w_r_yom) + np.exp(w_r_yfm)))
    w = w_b1 / p_norm
    f0 = float(1.0 - oo1 / (1.0 + np.exp(-b0)))

    B, T = x.shape
    key = (B, T, time_lag, L, K, w, b0, oo1)
    if key not in _CACHE:
        _CACHE[key] = _build(B, T, time_lag, L, K, w, b0, oo1, f0)
    nc = _CACHE[key]

    n_cores = 8
    in_maps = [{"x": x} for _ in range(n_cores)]
    r = run_bass_kernel_spmd(nc, in_maps, core_ids=list(range(n_cores)), trace=trace)
    res = r.results[0]["out"]  # [R, 4]

    outs = []
    for j in range(4):
        full = np.zeros((B, 1), dtype=np.float32)
        full[time_lag:, 0] = res[:, j]
        outs.append(full)
    return tuple(outs), r.exec_time_ns


def kernel(**inputs):
    outs, _ = run(inputs)
    return outs

